# revision 14
# baseline (speedup 1.0000x reference)
"""Causal attention with RoPE on 8 Trainium2 NeuronCores.

Tensor-parallel over heads: core c owns heads [2c, 2c+2). Each core computes
its heads' Q/K/V projections, RoPE, causal attention in a transposed layout
(keys on partitions), and a partial output projection through its slice of
Wo. The 8 partial outputs are summed on the host.

v2 layout notes:
  - All matmul operands are fp16 (1 cycle/row on the PE like fp32r, half the
    DMA bytes and SBUF of f32, and 2x DVE throughput on pure-fp16 ops).
    PSUM accumulation stays fp32.
  - Softmax denominators never touch the PE: exp tiles accumulate into a
    per-(head, query-block) fp16 accumulator on the DVE (scaled by 1/16 for
    headroom), a gpsimd partition_all_reduce produces the key-dim sum on all
    partitions, and one fused scalar_tensor_tensor (po/16)/dsum performs the
    normalization per head.
  - The emission is software-pipelined two phases deep: projections for
    phase i+2 are emitted between attention i and i+1, and the Wo matmuls
    for phase i-1 are interleaved into attention i's rounds so the PE always
    has work while the Activation engine chews exps.
  - DMA queues: weights/tables on gpsimd (SWDGE), x loads on sync, output
    stores + Wo PSUM drains on gpsimd so the Activation queue only runs exp.
"""
import numpy as np

import concourse.bacc as bacc
import concourse.bass as bass
import concourse.bass_isa as bass_isa
import concourse.tile as tile
import concourse.mybir as mybir
from concourse.bass_utils import run_bass_kernel_spmd

AF = mybir.ActivationFunctionType
ALU = mybir.AluOpType
F32 = mybir.dt.float32
F16 = mybir.dt.float16

P = 128            # partitions
DH = 128           # head dim
D = 2048           # d_model
S = 2048           # sequence length
B = 2              # batch
NCORES = 8
HL = 2             # heads per core
LF = HL * DH       # 256 local head features
KC = D // P        # 16 d_model chunks
NCB = S // 512     # 4 column blocks of 512 positions per batch
NKB = S // P       # 16 key blocks per batch
NNT = D // 512     # 4 output column tiles
ROWS = B * S
SCALE = float(1.0 / np.sqrt(DH))
DSC = 0.0625       # denominator accumulation scale (fp16 headroom)

_PROG = None


def _emit(nc, sbp, psp, t):
    xT, wqT, wkT, wvT, woT, cosT, sinT, bandT, identT, out = (
        t["xT"], t["wqT"], t["wkT"], t["wvT"], t["woT"], t["cosT"], t["sinT"],
        t["bandT"], t["identT"], t["out"])

    # ---------------- constants ----------------
    wq = sbp.tile([P, KC * LF], F16, name="wq")
    wk = sbp.tile([P, KC * LF], F16, name="wk")
    wv = sbp.tile([P, KC * LF], F16, name="wv")
    wo = sbp.tile([P, HL * D], F16, name="wo")
    cos = sbp.tile([DH, S], F32, name="cos")
    sin = sbp.tile([DH, S], F32, name="sin")
    band = sbp.tile([P, 640], F16, name="band")
    ident = sbp.tile([P, P], F16, name="ident")

    # const loads ordered by first use, all on the gpsimd (SWDGE) queue so
    # the Activation queue never issues DMA
    for g in range(4):
        gk = slice(g * 4 * P, (g + 1) * 4 * P)
        nc.gpsimd.dma_start(
            out=wv[:, g * 4 * LF:(g + 1) * 4 * LF],
            in_=wvT[gk, :].rearrange("(kc p) f -> p kc f", p=P))
        nc.gpsimd.dma_start(
            out=wk[:, g * 4 * LF:(g + 1) * 4 * LF],
            in_=wkT[gk, :].rearrange("(kc p) f -> p kc f", p=P))
        nc.gpsimd.dma_start(
            out=wq[:, g * 4 * LF:(g + 1) * 4 * LF],
            in_=wqT[gk, :].rearrange("(kc p) f -> p kc f", p=P))
    nc.gpsimd.dma_start(out=cos, in_=cosT[:, :])
    nc.gpsimd.dma_start(out=sin, in_=sinT[:, :])
    nc.gpsimd.dma_start(out=band, in_=bandT[:, :])
    nc.gpsimd.dma_start(out=ident, in_=identT[:, :])
    for half in range(2):
        for h in range(HL):
            nc.gpsimd.dma_start(
                out=wo[:, h * D + half * 1024: h * D + (half + 1) * 1024],
                in_=woT[h * P:(h + 1) * P, half * 1024:(half + 1) * 1024])

    # ---------------- per-batch tiles ----------------
    bt = {}

    def get_bt(b):
        if b not in bt:
            bt[b] = dict(
                qt=sbp.tile([P, HL * S], F16, name=f"qt{b}", tag="qt", bufs=2),
                kt=sbp.tile([P, HL * S], F16, name=f"kt{b}", tag="kt", bufs=2),
                vsb=sbp.tile([P, NKB * LF], F16, name=f"v{b}", tag="v",
                             bufs=2),
                ot=sbp.tile([P, HL * S], F16, name=f"ot{b}", tag="ot",
                            bufs=2),
            )
        return bt[b]

    # ---------------- projection phase ----------------
    def emit_proj(b, s):
        T = get_bt(b)
        xtg = []
        for g in range(4):
            xt = sbp.tile([P, 4 * 512], F16, name=f"xt{b}_{s}_{g}", tag="xt",
                          bufs=6)
            src = xT[g * 4 * P:(g + 1) * 4 * P,
                     b * S + s * 512: b * S + (s + 1) * 512]
            nc.sync.dma_start(
                out=xt, in_=src.rearrange("(kc p) s -> p kc s", p=P))
            xtg.append(xt)
        xts = [xtg[kc // 4][:, (kc % 4) * 512:(kc % 4 + 1) * 512]
               for kc in range(KC)]

        # V projection, transposed like Q/K (half the matmuls of the
        # natural layout), then PE tile-transposes into [keys, dh] vsb
        vt = sbp.tile([P, HL * 512], F16, name=f"vt{b}_{s}", tag="vt",
                      bufs=2)
        for h in range(HL):
            pvt = psp.tile([P, 512], F32, name=f"pv{h}_{b}_{s}", tag="ps",
                           bufs=4)
            for kc in range(KC):
                nc.tensor.matmul(
                    pvt,
                    lhsT=wv[:, kc * LF + h * DH: kc * LF + (h + 1) * DH],
                    rhs=xts[kc], start=(kc == 0), stop=(kc == KC - 1))
            nc.scalar.copy(vt[:, h * 512:(h + 1) * 512], pvt)
        for h in range(HL):
            for r in range(4):
                ptr = psp.tile([P, DH], F16, name=f"pt{h}{r}_{b}_{s}",
                               tag="ps", bufs=4)
                nc.tensor.transpose(
                    ptr, vt[:, h * 512 + r * P: h * 512 + (r + 1) * P],
                    ident)
                kb = s * 4 + r
                dst = T["vsb"][:, kb * LF + h * DH: kb * LF + (h + 1) * DH]
                if r % 2 == 0:
                    nc.scalar.copy(dst, ptr)
                else:
                    nc.vector.tensor_copy(dst, ptr)

        # K then Q projections, serial per head, RoPE drain on the DVE
        cs = slice(s * 512, (s + 1) * 512)
        for key, tag, wsb, dst in (("k", "pd", wk, T["kt"]),
                                   ("q", "po", wq, T["qt"])):
            for h in range(HL):
                pq = psp.tile([P, 512], F32, name=f"p{key}{h}_{b}_{s}",
                              tag=tag, bufs=2)
                for kc in range(KC):
                    nc.tensor.matmul(
                        pq,
                        lhsT=wsb[:, kc * LF + h * DH: kc * LF + (h + 1) * DH],
                        rhs=xts[kc], start=(kc == 0), stop=(kc == KC - 1))
                dsl = dst[:, h * S + s * 512: h * S + (s + 1) * 512]
                ra = sbp.tile([P, 512], F32, name=f"ra{b}_{s}_{key}{h}",
                              tag="ra", bufs=2)
                nc.vector.tensor_mul(ra, pq, cos[:, cs])
                nc.vector.tensor_mul(dsl[0:64, :], pq[64:128, :],
                                     sin[0:64, cs])
                nc.vector.tensor_mul(dsl[64:128, :], pq[0:64, :],
                                     sin[64:128, cs])
                nc.vector.tensor_add(dsl, dsl, ra)

    # ---------------- Wo phase (fed incrementally into attention) --------
    def make_wo_feed(b, qj):
        T = get_bt(b)
        steps = []
        sts = {}

        def step(qc, nt):
            def run():
                if nt == 0:
                    sts[qc] = sbp.tile([P, NNT * 512], F16,
                                       name=f"st{b}_{qc}", tag="st", bufs=3)
                st = sts[qc]
                pw = psp.tile([P, 512], F32, name=f"pw{b}_{qc}_{nt}",
                              tag="ps", bufs=4)
                for h in range(HL):
                    nc.tensor.matmul(
                        pw,
                        lhsT=T["ot"][:, h * S + qc * P: h * S + (qc + 1) * P],
                        rhs=wo[:, h * D + nt * 512: h * D + (nt + 1) * 512],
                        start=(h == 0), stop=(h == HL - 1))
                # gpsimd cannot read PSUM; alternate the drain between the
                # Activation and DVE engines
                if nt % 2 == 0:
                    nc.scalar.copy(st[:, nt * 512:(nt + 1) * 512], pw)
                else:
                    nc.vector.tensor_copy(st[:, nt * 512:(nt + 1) * 512], pw)
                if nt == NNT - 1:
                    nc.gpsimd.dma_start(
                        out=out[b * S + qc * P: b * S + (qc + 1) * P, :],
                        in_=st)
            return run

        for qc in range(4 * qj, 4 * qj + 4):
            for nt in range(NNT):
                steps.append(step(qc, nt))
        return steps

    # ---------------- attention phase ----------------
    def emit_attn(b, qj, wo_steps):
        T = get_bt(b)
        qt, kt, vsb, ot = T["qt"], T["kt"], T["vsb"], T["ot"]
        nkb = 4 * qj + 4
        po = {}
        dacc = {}
        exs = {}
        for h in range(HL):
            po[h] = psp.tile([P, 512], F32, name=f"po{b}_{h}_{qj}",
                             tag="po", bufs=2)
            dacc[h] = sbp.tile([P, 512], F16, name=f"da{b}_{h}_{qj}",
                               tag="da", bufs=2)

        def _off(kb):
            return max(0, kb - 4 * qj) * P

        def emit_sc(h, kb):
            off = _off(kb)
            pss = psp.tile([P, 512], F32, name=f"pss{b}_{h}_{qj}_{kb}",
                           tag="ps", bufs=4)
            nc.tensor.matmul(
                pss[:, off:512],
                lhsT=kt[:, h * S + kb * P: h * S + (kb + 1) * P],
                rhs=qt[:, h * S + qj * 512 + off: h * S + (qj + 1) * 512],
                start=True, stop=True)
            ex = sbp.tile([P, 512], F16, name=f"ex{b}_{h}_{qj}_{kb}",
                          tag="ex", bufs=6)
            nc.scalar.activation(ex[:, off:512], pss[:, off:512], AF.Exp,
                                 scale=SCALE)
            if kb >= 4 * qj:
                # upper-triangle mask on the diagonal 128-block
                nc.vector.tensor_mul(
                    ex[:, off:off + P], ex[:, off:off + P], band[:, 512:640])
            # denominator accumulation (never touches the PE); fp16 sums of
            # raw exp values stay well under fp16 max (<= ~16 * e^7)
            if kb == 0:
                nc.vector.tensor_copy(dacc[h], ex)
            else:
                nc.vector.tensor_add(
                    dacc[h][:, off:512], ex[:, off:512],
                    dacc[h][:, off:512])
            exs[(h, kb)] = ex

        def emit_av(h, kb, last):
            off = _off(kb)
            nc.tensor.matmul(
                po[h][:, off:512],
                lhsT=vsb[:, kb * LF + h * DH: kb * LF + h * DH + DH],
                rhs=exs[(h, kb)][:, off:512], start=(kb == 0), stop=last)

        emit_sc(0, 0)
        emit_sc(1, 0)
        wo_i = 0
        for kb in range(nkb):
            for h in range(HL):
                if kb + 1 < nkb:
                    emit_sc(h, kb + 1)
                emit_av(h, kb, last=(kb == nkb - 1))
            # interleave Wo work for the previous block so the PE keeps
            # busy while the Act engine runs this round's exps
            want = ((kb + 1) * len(wo_steps)) // nkb
            while wo_i < want:
                wo_steps[wo_i]()
                wo_i += 1

        for h in range(HL):
            dsum = sbp.tile([P, 512], F16, name=f"ds{b}_{h}_{qj}",
                            tag="ds", bufs=2)
            nc.gpsimd.partition_all_reduce(
                dsum, dacc[h], channels=P, reduce_op=bass_isa.ReduceOp.add)
            rds = sbp.tile([P, 512], F16, name=f"rd{b}_{h}_{qj}",
                           tag="rd", bufs=2)
            with nc.allow_low_precision(reason="fp16 1/denom is 0.05% rel"):
                nc.vector.reciprocal(rds, dsum)
            nc.vector.tensor_mul(
                ot[:, h * S + qj * 512: h * S + (qj + 1) * 512],
                po[h], rds)

    # ---------------- pipelined emission ----------------
    phases = [(b, s) for b in range(B) for s in range(NCB)]
    emit_proj(*phases[0])
    emit_proj(*phases[1])
    for i, (b, s) in enumerate(phases):
        wo_steps = make_wo_feed(*phases[i - 1]) if i > 0 else []
        emit_attn(b, s, wo_steps)
        if i + 2 < len(phases):
            emit_proj(*phases[i + 2])
    for stp in make_wo_feed(*phases[-1]):
        stp()


def _build(loop_n=0):
    nc = bacc.Bacc("TRN2", target_bir_lowering=False, debug=False)
    t = {}
    t["xT"] = nc.dram_tensor("xT", [D, ROWS], F16, kind="ExternalInput")
    t["wqT"] = nc.dram_tensor("wqT", [D, LF], F16, kind="ExternalInput")
    t["wkT"] = nc.dram_tensor("wkT", [D, LF], F16, kind="ExternalInput")
    t["wvT"] = nc.dram_tensor("wvT", [D, LF], F16, kind="ExternalInput")
    t["woT"] = nc.dram_tensor("woT", [LF, D], F16, kind="ExternalInput")
    t["cosT"] = nc.dram_tensor("cosT", [DH, S], F32, kind="ExternalInput")
    t["sinT"] = nc.dram_tensor("sinT", [DH, S], F32, kind="ExternalInput")
    t["bandT"] = nc.dram_tensor("bandT", [P, 640], F16, kind="ExternalInput")
    t["identT"] = nc.dram_tensor("identT", [P, P], F16, kind="ExternalInput")
    t["out"] = nc.dram_tensor("out", [ROWS, D], F16, kind="ExternalOutput")
    with tile.TileContext(nc) as tc:
        with tc.tile_pool(name="sb", bufs=1) as sbp, \
             tc.tile_pool(name="ps", bufs=4, space="PSUM") as psp:
            if loop_n:
                with tc.For_i(0, loop_n, 1,
                              hint_engines=(mybir.EngineType.PE,
                                            mybir.EngineType.Activation,
                                            mybir.EngineType.DVE)):
                    _emit(nc, sbp, psp, t)
            else:
                _emit(nc, sbp, psp, t)
    nc.compile()
    return nc


def _tables():
    half = np.arange(0, DH, 2, dtype=np.float32) / np.float32(DH)
    inv_freq = (np.float32(1.0) / (np.float32(10000.0) ** half)).astype(np.float32)
    pos = np.arange(S, dtype=np.float32)
    freqs = np.outer(pos, inv_freq).astype(np.float32)        # [S, 64]
    emb = np.concatenate([freqs, freqs], axis=1)              # [S, DH]
    cosT = np.ascontiguousarray(np.cos(emb).T).astype(np.float32)
    sinT = np.sin(emb).T.astype(np.float32).copy()
    sinT[0:DH // 2, :] *= np.float32(-1.0)                    # pre-signed
    sinT = np.ascontiguousarray(sinT)
    # band[kl, c] = 1 iff c >= kl + 512; slice [512:640] is the upper-triangle
    # mask for a diagonal 128-block
    kl = np.arange(P)[:, None]
    c = np.arange(640)[None, :]
    bandT = (c >= kl + 512).astype(np.float16)
    identT = np.eye(P, dtype=np.float16)
    return cosT, sinT, bandT, identT


def _make_in_maps(inputs):
    q = np.asarray(inputs["query"], dtype=np.float32)
    Wq = np.asarray(inputs["Wq"], dtype=np.float32)
    Wk = np.asarray(inputs["Wk"], dtype=np.float32)
    Wv = np.asarray(inputs["Wv"], dtype=np.float32)
    Wo = np.asarray(inputs["Wo"], dtype=np.float32)
    xT = np.ascontiguousarray(q.reshape(ROWS, D).T.astype(np.float16))
    cosT, sinT, bandT, identT = _tables()
    in_maps = []
    for ci in range(NCORES):
        rs = slice(ci * LF, (ci + 1) * LF)
        in_maps.append({
            "xT": xT,
            "wqT": np.ascontiguousarray(Wq[rs, :].T.astype(np.float16)),
            "wkT": np.ascontiguousarray(Wk[rs, :].T.astype(np.float16)),
            "wvT": np.ascontiguousarray(Wv[rs, :].T.astype(np.float16)),
            "woT": np.ascontiguousarray(Wo[:, rs].T.astype(np.float16)),
            "cosT": cosT, "sinT": sinT, "bandT": bandT, "identT": identT,
        })
    return in_maps


def _run(inputs, trace=False, **kw):
    global _PROG
    if _PROG is None:
        _PROG = _build()
    in_maps = _make_in_maps(inputs)
    res = run_bass_kernel_spmd(_PROG, in_maps, core_ids=list(range(NCORES)),
                               trace=trace, **kw)
    acc = np.zeros((ROWS, D), np.float64)
    for r in res.results:
        acc += r["out"].astype(np.float32)
    return acc.astype(np.float32).reshape(B, S, D), res


def kernel(query, Wq, Wk, Wv, Wo):
    out, _ = _run(dict(query=query, Wq=Wq, Wk=Wk, Wv=Wv, Wo=Wo))
    return out
